# revision 13
# baseline (speedup 1.0000x reference)
"""CondConv2d Trainium2 kernel.

Problem: per-sample 3x3 'same' conv, B=16, CIN=COUT=32, H=W=256, with
per-sample weights mixed from 8 experts by routing weights.

Strategy (v3, block-diagonal 4-sample packing):
- Sharding: each of the 8 cores takes 4 samples x 128 rows (sample-group
  c//2, row-half c%2), so 4 samples fill all 128 PE partitions.
- Host: routing matmuls produce per-sample conv weights + bias. Weights are
  packed as 9 block-diagonal [128,128] bf16 stationaries (one per kernel
  tap (kh,kw)); block (s,s) holds sample s's [cin,cout] matrix, off-diagonal
  blocks are zero so cross-sample contraction terms vanish exactly.
- Device: conv = 9 PSUM-accumulated bf16 matmuls per [128, 512] output tile
  (K = 4 samples x 32 cin = 128, M = 4 samples x 32 cout = 128). The kh/kw
  shifts are pure free-dim offsets into the input strip, so NO shifted input
  replicas are needed: input is DMA'd once, 9 taps read it at offsets.
  2.25 PE cycles per output pixel vs 3.0 for the 32-wide layout, and every
  engine (DMA, ACT evac) runs 128 partitions wide.
- Bias: added during PSUM->SBUF evacuation via ACT's activation bias path
  (out = in + bias[128,1]), so no ones-row contraction row is needed.
- PSUM: chunks of 8 output rows = [128, 2048] fp32 = 4 banks, double
  buffered. ACT evacuates, GpSimd (SWDGE) dispatches output DMA.
"""

import numpy as np

B, CIN, H, W = 16, 32, 256, 256
COUT, KH, KW = 32, 3, 3
NCORES = 8
SPC = 4  # samples per core
HH = H // 2  # rows per core (row-half)

RP = 260  # padded row pitch (cols 1..256 hold x, 0/257 zero, 258-259 slack)
PADROWS = HH + 2  # 130 rows staged per core
STRIP_OUT = 32  # output rows per strip
STRIP_ROWS = STRIP_OUT + 2  # 34
SFREE = STRIP_ROWS * RP  # 8840
NSTRIPS = HH // STRIP_OUT  # 4
CHUNK_OUT = 8  # output rows per PSUM chunk
NCHUNK = STRIP_OUT // CHUNK_OUT  # 4
NT = CHUNK_OUT // 2  # matmul N-tiles per chunk (N=512 = 2 rows)
NTAP = KH * KW  # 9 kernel taps
PSUM_BUFS = 2

_cache = {}


def _build():
    import concourse.bacc as bacc
    import concourse.mybir as mybir
    from concourse.tile import TileContext

    BF16 = mybir.dt.bfloat16
    F32 = mybir.dt.float32

    nc = bacc.Bacc(name="condconv")
    x_d = nc.dram_tensor("xp", [SPC, CIN, PADROWS, RP], BF16, kind="ExternalInput")
    w_d = nc.dram_tensor("wt", [128, NTAP * 128], BF16, kind="ExternalInput")
    b_d = nc.dram_tensor("bias", [128, 1], F32, kind="ExternalInput")
    y_d = nc.dram_tensor("y", [SPC, COUT, HH, W], F32, kind="ExternalOutput")

    with TileContext(nc) as tc:
        with (
            tc.tile_pool(name="strip", bufs=4) as strip_pool,
            tc.tile_pool(name="wtp", bufs=1) as wt_pool,
            tc.tile_pool(name="stage", bufs=6) as stage_pool,
            tc.tile_pool(name="psum", bufs=PSUM_BUFS, space="PSUM") as psum_pool,
        ):
            wt = wt_pool.tile([128, NTAP * 128], BF16)
            # split: warmup slice (cols 0:512) lands first so PE ramps ASAP
            nc.sync.dma_start(out=wt[:, 0:512], in_=w_d[:, 0:512])
            nc.sync.dma_start(out=wt[:, 512:], in_=w_d[:, 512:])
            bias = wt_pool.tile([128, 1], F32)
            nc.sync.dma_start(out=bias, in_=b_d[:, :])

            def load_strip(s):
                r0 = s * STRIP_OUT
                strip = strip_pool.tile([128, SFREE], BF16, name="strip", tag="strip")
                s3 = strip.rearrange("p (y u) -> p y u", y=STRIP_ROWS)
                # strip 0 gates kernel start: load it in row-pieces so the
                # first chunk's matmuls begin after ~10 rows, not all 34
                pieces = [(0, 4), (4, 6), (10, 24)] if s == 0 else [(0, STRIP_ROWS)]
                for lo, n in pieces:
                    for smp in range(SPC):
                        nc.sync.dma_start(
                            out=s3[32 * smp : 32 * smp + 32, lo : lo + n],
                            in_=x_d[smp, :, r0 + lo : r0 + lo + n, :],
                        )
                return s3

            def emit_chunk(s3, r0, t0, nrows):
                """One PSUM chunk: output rows t0..t0+nrows (strip-local)."""
                ntiles = nrows // 2
                # uniform PSUM tile size so the 2-buf rotation fits 8 banks
                ps = psum_pool.tile([128, NT * 512], F32, name="ps", tag="ps")
                # tap-major: consecutive matmuls share each stationary
                for p in range(NTAP):
                    kh, kw = divmod(p, KW)
                    for nt in range(ntiles):
                        t = t0 + nt * 2
                        nc.tensor.matmul(
                            ps[:, nt * 512 : (nt + 1) * 512],
                            wt[:, p * 128 : (p + 1) * 128],
                            s3[:, t + kh : t + kh + 2, kw : kw + W],
                            start=(p == 0),
                            stop=(p == NTAP - 1),
                        )
                stage = stage_pool.tile([128, ntiles * 512], F32)
                # PSUM->SBUF evac with fused per-partition bias add
                nc.scalar.add(out=stage, in_=ps[:, 0 : ntiles * 512], add=bias)
                lo = r0 + t0
                nc.gpsimd.dma_start(
                    out=y_d[:, :, lo : lo + nrows, :],
                    in_=stage.rearrange("p (r w) -> p r w", r=nrows),
                )

            def emit_chunks(s, s3, last=False):
                r0 = s * STRIP_OUT
                for q in range(NCHUNK):
                    if last and q == NCHUNK - 1:
                        # final chunk at 2-row granularity: the post-matmul
                        # evac+DMA drain shrinks from ~8.5us to ~2.5us
                        for t0 in range(q * CHUNK_OUT, (q + 1) * CHUNK_OUT, 2):
                            emit_chunk(s3, r0, t0, 2)
                    else:
                        emit_chunk(s3, r0, q * CHUNK_OUT, CHUNK_OUT)

            # PE warmup while strip 0 loads: throwaway matmuls on the weight
            # tile ramp the PE clock (0.65/1.2 GHz p-states -> 2.4 GHz after
            # ~3us of continuous busy) so real matmuls start at full speed
            wps = psum_pool.tile([128, NT * 512], F32, name="ps", tag="ps")
            for i in range(6):
                nc.tensor.matmul(
                    wps[:, 0:512],
                    wt[:, 0:128],
                    wt[:, 0:512],
                    start=(i == 0),
                    stop=(i == 5),
                )

            # software pipeline: strip s+1's loads land before strip s's
            # matmuls consume strip s
            pending = None
            for s in range(NSTRIPS):
                s3 = load_strip(s)
                if pending is not None:
                    emit_chunks(pending[0], pending[1])
                pending = (s, s3)
            emit_chunks(pending[0], pending[1], last=True)
    nc.compile()
    return nc


def kernel(x, routing_weights, expert_weight, expert_bias):
    import ml_dtypes
    from concourse import bass_utils

    bf16 = ml_dtypes.bfloat16
    x = np.ascontiguousarray(x, dtype=np.float32)
    routing_weights = np.asarray(routing_weights, dtype=np.float32)
    expert_weight = np.asarray(expert_weight, dtype=np.float32)
    expert_bias = np.asarray(expert_bias, dtype=np.float32)

    # Host: routing mix (trivial flops) + weight/input repacking.
    w_all = routing_weights @ expert_weight  # [B, COUT*CIN*KH*KW]
    bias = routing_weights @ expert_bias  # [B, COUT]
    w4 = w_all.reshape(B, COUT, CIN, KH, KW)

    # Per group of 4 samples: 9 block-diagonal stationaries, k-major layout
    # wtb[k=(s,ci), p=(kh,kw), m=(s,co)] = w4[sample s, co, ci, kh, kw]
    ngrp = B // SPC
    wtb = np.zeros((ngrp, 128, NTAP, 128), np.float32)
    # w4 grouped: [ngrp, SPC, COUT, CIN, KH, KW] -> [g, (s ci), (kh kw), co]
    w4g = w4.reshape(ngrp, SPC, COUT, CIN, KH, KW)
    for s in range(SPC):
        blk = w4g[:, s].transpose(0, 2, 3, 4, 1)  # [g, CIN, KH, KW, COUT]
        wtb[:, 32 * s : 32 * s + 32, :, 32 * s : 32 * s + 32] = blk.reshape(
            ngrp, CIN, NTAP, COUT
        )
    wtb = wtb.reshape(ngrp, 128, NTAP * 128).astype(bf16)
    biasb = bias.reshape(ngrp, SPC * COUT, 1).astype(np.float32)

    # Padded input: col c in 1..256 holds x col c-1; row r in 1..256 holds
    # x row r-1; everything else zero.
    xp = np.zeros((B, CIN, H + 2, RP), bf16)
    xp[:, :, 1 : 1 + H, 1 : 1 + W] = x

    if "nc" not in _cache:
        _cache["nc"] = _build()
    nc = _cache["nc"]

    # core c: sample group c//2, row half c%2 (row slab 128*h .. 128*h+129)
    in_maps = []
    for c in range(NCORES):
        g, h = divmod(c, 2)
        slab = xp[g * SPC : (g + 1) * SPC, :, h * HH : h * HH + PADROWS, :]
        in_maps.append(
            {
                "xp": np.ascontiguousarray(slab),
                "wt": wtb[g],
                "bias": biasb[g],
            }
        )
    import os

    trace = bool(int(os.environ.get("CONDCONV_TRACE", "0")))
    res = bass_utils.run_bass_kernel_spmd(
        nc, in_maps, core_ids=list(range(NCORES)), trace=trace
    )
    _cache["last_results"] = res
    y = np.empty((B, COUT, H, W), np.float32)
    for c in range(NCORES):
        g, h = divmod(c, 2)
        y[g * SPC : (g + 1) * SPC, :, h * HH : (h + 1) * HH, :] = res.results[c]["y"]
    return y


# revision 44
# speedup vs baseline: 1.0433x; 1.0433x over previous
"""CondConv2d Trainium2 kernel.

Problem: per-sample 3x3 'same' conv, B=16, CIN=COUT=32, H=W=256, with
per-sample weights mixed from 8 experts by routing weights.

Strategy (v3, block-diagonal 4-sample packing):
- Sharding: each of the 8 cores takes 4 samples x 128 rows (sample-group
  c//2, row-half c%2), so 4 samples fill all 128 PE partitions.
- Host: routing matmuls produce per-sample conv weights + bias. Weights are
  packed as 9 block-diagonal [128,128] bf16 stationaries (one per kernel
  tap (kh,kw)); block (s,s) holds sample s's [cin,cout] matrix, off-diagonal
  blocks are zero so cross-sample contraction terms vanish exactly.
- Device: conv = 9 PSUM-accumulated bf16 matmuls per [128, 512] output tile
  (K = 4 samples x 32 cin = 128, M = 4 samples x 32 cout = 128). The kh/kw
  shifts are pure free-dim offsets into the input strip, so NO shifted input
  replicas are needed: input is DMA'd once, 9 taps read it at offsets.
  2.25 PE cycles per output pixel vs 3.0 for the 32-wide layout, and every
  engine (DMA, ACT evac) runs 128 partitions wide.
- Bias: added during PSUM->SBUF evacuation via ACT's activation bias path
  (out = in + bias[128,1]), so no ones-row contraction row is needed.
- PSUM: chunks of 8 output rows = [128, 2048] fp32 = 4 banks, double
  buffered. ACT evacuates, GpSimd (SWDGE) dispatches output DMA.
"""

import numpy as np

B, CIN, H, W = 16, 32, 256, 256
COUT, KH, KW = 32, 3, 3
NCORES = 8
SPC = 4  # samples per core
HH = H // 2  # rows per core (row-half)

RP = 260  # padded row pitch (cols 1..256 hold x, 0/257 zero, 258-259 slack)
PADROWS = HH + 2  # 130 rows staged per core
STRIP_OUT = 32  # output rows per strip
STRIP_ROWS = STRIP_OUT + 2  # 34
SFREE = STRIP_ROWS * RP  # 8840
NSTRIPS = HH // STRIP_OUT  # 4
CHUNK_OUT = 8  # output rows per PSUM chunk
NCHUNK = STRIP_OUT // CHUNK_OUT  # 4
NT = CHUNK_OUT // 2  # matmul N-tiles per chunk (N=512 = 2 rows)
NTAP = KH * KW  # 9 kernel taps
PSUM_BUFS = 2

_cache = {}


def _build():
    import concourse.bacc as bacc
    import concourse.mybir as mybir
    from concourse.tile import TileContext

    BF16 = mybir.dt.bfloat16
    F32 = mybir.dt.float32

    nc = bacc.Bacc(name="condconv")
    x_d = nc.dram_tensor("xp", [SPC, CIN, PADROWS, RP], BF16, kind="ExternalInput")
    w_d = nc.dram_tensor("wt", [128, NTAP * 128], BF16, kind="ExternalInput")
    b_d = nc.dram_tensor("bias", [128, 1], F32, kind="ExternalInput")
    y_d = nc.dram_tensor("y", [SPC, COUT, HH, W], F32, kind="ExternalOutput")

    with TileContext(nc) as tc:
        with (
            tc.tile_pool(name="strip", bufs=4) as strip_pool,
            tc.tile_pool(name="wtp", bufs=1) as wt_pool,
            tc.tile_pool(name="stage", bufs=6) as stage_pool,
            tc.tile_pool(name="psum", bufs=PSUM_BUFS, space="PSUM") as psum_pool,
        ):
            wt = wt_pool.tile([128, NTAP * 128], BF16)
            bias = wt_pool.tile([128, 1], F32)

            def load_strip(s):
                r0 = s * STRIP_OUT
                strip = strip_pool.tile([128, SFREE], BF16, name="strip", tag="strip")
                s3 = strip.rearrange("p (y u) -> p y u", y=STRIP_ROWS)
                # strip 0 gates kernel start: load it in row-pieces so the
                # first chunk's matmuls begin after ~10 rows, not all 34
                if s == 0:
                    # startup interleave on the serial DMA bus: piece 1 (rows
                    # 0-3, gates the first matmul) first, weight slices
                    # between the later pieces (tap p's matmuls only need
                    # wt[:, 512p:...], tracked at region granularity); bias
                    # (512B, needed by the first evac ~7us in) rides second
                    nc.sync.dma_start(out=s3[:, 0:4], in_=x_d[:, :, r0 : r0 + 4, :])
                    nc.sync.dma_start(out=bias, in_=b_d[:, :])
                    nc.sync.dma_start(out=wt[:, 0:512], in_=w_d[:, 0:512])
                    nc.sync.dma_start(
                        out=s3[:, 4:10], in_=x_d[:, :, r0 + 4 : r0 + 10, :]
                    )
                    nc.sync.dma_start(out=wt[:, 512:], in_=w_d[:, 512:])
                    nc.sync.dma_start(
                        out=s3[:, 10:18], in_=x_d[:, :, r0 + 10 : r0 + 18, :]
                    )
                    nc.sync.dma_start(
                        out=s3[:, 18:34], in_=x_d[:, :, r0 + 18 : r0 + 34, :]
                    )
                else:
                    # one fused DMA (partition dim = samples x cin): a single
                    # HWDGE generation instead of four serialized ones
                    nc.sync.dma_start(
                        out=s3[:, :],
                        in_=x_d[:, :, r0 : r0 + STRIP_ROWS, :],
                    )
                return s3

            def emit_chunk(
                s3, r0, t0, nrows, out_sp=False, evac_dve=False, stag="st"
            ):
                """One PSUM chunk: output rows t0..t0+nrows (strip-local)."""
                ntiles = (nrows + 1) // 2
                # uniform PSUM tile size so the 2-buf rotation fits 8 banks
                ps = psum_pool.tile([128, NT * 512], F32, name="ps", tag="ps")
                # tap-major: consecutive matmuls share each stationary
                for p in range(NTAP):
                    kh, kw = divmod(p, KW)
                    for nt in range(ntiles):
                        t = t0 + nt * 2
                        r = min(2, nrows - nt * 2)
                        nc.tensor.matmul(
                            ps[:, nt * 512 : nt * 512 + r * 256],
                            wt[:, p * 128 : (p + 1) * 128],
                            s3[:, t + kh : t + kh + r, kw : kw + W],
                            start=(p == 0),
                            stop=(p == NTAP - 1),
                        )
                fsz = nrows * 256
                stage = stage_pool.tile([128, fsz], F32, name="st", tag="st")
                # PSUM->SBUF evac with fused per-partition bias add; the
                # drain path alternates ACT/DVE so consecutive evacs don't
                # serialize behind each other's output-DMA semaphores
                if evac_dve:
                    nc.vector.tensor_scalar_add(
                        out=stage, in0=ps[:, 0:fsz], scalar1=bias
                    )
                else:
                    nc.scalar.add(out=stage, in_=ps[:, 0:fsz], add=bias)
                lo = r0 + t0
                dma = {0: nc.gpsimd.dma_start, 1: nc.sync.dma_start,
                       2: nc.scalar.dma_start}[int(out_sp)]
                dma(
                    out=y_d[:, :, lo : lo + nrows, :],
                    in_=stage.rearrange("p (r w) -> p r w", r=nrows),
                )

            def emit_chunks(s, s3, first=False, last=False):
                r0 = s * STRIP_OUT
                for q in range(NCHUNK):
                    if first and q == 0:
                        # 2-row minis: each gated only on its own input rows,
                        # so matmuls start as soon as the 4-row piece lands
                        for t0 in range(0, CHUNK_OUT, 2):
                            emit_chunk(s3, r0, t0, 2)
                    elif last and q == NCHUNK - 1:
                        emit_chunk(s3, r0, q * CHUNK_OUT, 4)
                        emit_chunk(
                            s3, r0, q * CHUNK_OUT + 4, 4,
                            out_sp=True, evac_dve=True,
                        )
                    else:
                        emit_chunk(s3, r0, q * CHUNK_OUT, CHUNK_OUT)

            # PE warmup while strip 0 loads: throwaway matmuls on the weight
            # tile ramp the PE clock (0.65/1.2 GHz p-states -> 2.4 GHz after
            # ~3us of continuous busy) so real matmuls start at full speed
            # memset-sourced warmup: DVE fills a scratch tile in ~0.7us (no
            # DMA latency chain), so PE ramping starts at ~0.8us instead of
            # waiting ~3us for the first weight DMA to land
            warm = wt_pool.tile([128, 512], BF16)
            nc.vector.memset(warm, 0.0)
            wps = psum_pool.tile([128, NT * 512], F32, name="ps", tag="ps")
            NWARM = 5
            for i in range(NWARM):
                nc.tensor.matmul(
                    wps[:, 0:512],
                    warm[:, 0:128],
                    warm[:, :],
                    start=(i == 0),
                    stop=(i == NWARM - 1),
                )

            # software pipeline: strip s+1's loads land before strip s's
            # matmuls consume strip s
            pending = None
            for s in range(NSTRIPS):
                s3 = load_strip(s)
                if pending is not None:
                    emit_chunks(pending[0], pending[1], first=(pending[0] == 0))
                pending = (s, s3)
            emit_chunks(pending[0], pending[1], last=True)
    nc.compile()
    return nc


def kernel(x, routing_weights, expert_weight, expert_bias):
    import ml_dtypes
    from concourse import bass_utils

    bf16 = ml_dtypes.bfloat16
    x = np.ascontiguousarray(x, dtype=np.float32)
    routing_weights = np.asarray(routing_weights, dtype=np.float32)
    expert_weight = np.asarray(expert_weight, dtype=np.float32)
    expert_bias = np.asarray(expert_bias, dtype=np.float32)

    # Host: routing mix (trivial flops) + weight/input repacking.
    w_all = routing_weights @ expert_weight  # [B, COUT*CIN*KH*KW]
    bias = routing_weights @ expert_bias  # [B, COUT]
    w4 = w_all.reshape(B, COUT, CIN, KH, KW)

    # Per group of 4 samples: 9 block-diagonal stationaries, k-major layout
    # wtb[k=(s,ci), p=(kh,kw), m=(s,co)] = w4[sample s, co, ci, kh, kw]
    ngrp = B // SPC
    wtb = np.zeros((ngrp, 128, NTAP, 128), np.float32)
    # w4 grouped: [ngrp, SPC, COUT, CIN, KH, KW] -> [g, (s ci), (kh kw), co]
    w4g = w4.reshape(ngrp, SPC, COUT, CIN, KH, KW)
    for s in range(SPC):
        blk = w4g[:, s].transpose(0, 2, 3, 4, 1)  # [g, CIN, KH, KW, COUT]
        wtb[:, 32 * s : 32 * s + 32, :, 32 * s : 32 * s + 32] = blk.reshape(
            ngrp, CIN, NTAP, COUT
        )
    wtb = wtb.reshape(ngrp, 128, NTAP * 128).astype(bf16)
    biasb = bias.reshape(ngrp, SPC * COUT, 1).astype(np.float32)

    # Padded input: col c in 1..256 holds x col c-1; row r in 1..256 holds
    # x row r-1; everything else zero.
    xp = np.zeros((B, CIN, H + 2, RP), bf16)
    xp[:, :, 1 : 1 + H, 1 : 1 + W] = x

    if "nc" not in _cache:
        _cache["nc"] = _build()
    nc = _cache["nc"]

    # core c: sample group c//2, row half c%2 (row slab 128*h .. 128*h+129)
    in_maps = []
    for c in range(NCORES):
        g, h = divmod(c, 2)
        slab = xp[g * SPC : (g + 1) * SPC, :, h * HH : h * HH + PADROWS, :]
        in_maps.append(
            {
                "xp": np.ascontiguousarray(slab),
                "wt": wtb[g],
                "bias": biasb[g],
            }
        )
    import os

    trace = bool(int(os.environ.get("CONDCONV_TRACE", "0")))
    res = bass_utils.run_bass_kernel_spmd(
        nc, in_maps, core_ids=list(range(NCORES)), trace=trace
    )
    _cache["last_results"] = res
    y = np.empty((B, COUT, H, W), np.float32)
    for c in range(NCORES):
        g, h = divmod(c, 2)
        y[g * SPC : (g + 1) * SPC, :, h * HH : (h + 1) * HH, :] = res.results[c]["y"]
    return y


# revision 48
# speedup vs baseline: 1.0490x; 1.0055x over previous
"""CondConv2d Trainium2 kernel.

Problem: per-sample 3x3 'same' conv, B=16, CIN=COUT=32, H=W=256, with
per-sample weights mixed from 8 experts by routing weights.

Strategy (v3, block-diagonal 4-sample packing):
- Sharding: each of the 8 cores takes 4 samples x 128 rows (sample-group
  c//2, row-half c%2), so 4 samples fill all 128 PE partitions.
- Host: routing matmuls produce per-sample conv weights + bias. Weights are
  packed as 9 block-diagonal [128,128] bf16 stationaries (one per kernel
  tap (kh,kw)); block (s,s) holds sample s's [cin,cout] matrix, off-diagonal
  blocks are zero so cross-sample contraction terms vanish exactly.
- Device: conv = 9 PSUM-accumulated bf16 matmuls per [128, 512] output tile
  (K = 4 samples x 32 cin = 128, M = 4 samples x 32 cout = 128). The kh/kw
  shifts are pure free-dim offsets into the input strip, so NO shifted input
  replicas are needed: input is DMA'd once, 9 taps read it at offsets.
  2.25 PE cycles per output pixel vs 3.0 for the 32-wide layout, and every
  engine (DMA, ACT evac) runs 128 partitions wide.
- Bias: added during PSUM->SBUF evacuation via ACT's activation bias path
  (out = in + bias[128,1]), so no ones-row contraction row is needed.
- PSUM: chunks of 8 output rows = [128, 2048] fp32 = 4 banks, double
  buffered. ACT evacuates, GpSimd (SWDGE) dispatches output DMA.
"""

import numpy as np

B, CIN, H, W = 16, 32, 256, 256
COUT, KH, KW = 32, 3, 3
NCORES = 8
SPC = 4  # samples per core
HH = H // 2  # rows per core (row-half)

RP = 260  # padded row pitch (cols 1..256 hold x, 0/257 zero, 258-259 slack)
PADROWS = HH + 2  # 130 rows staged per core
STRIP_OUT = 32  # output rows per strip
STRIP_ROWS = STRIP_OUT + 2  # 34
SFREE = STRIP_ROWS * RP  # 8840
NSTRIPS = HH // STRIP_OUT  # 4
CHUNK_OUT = 8  # output rows per PSUM chunk
NCHUNK = STRIP_OUT // CHUNK_OUT  # 4
NT = CHUNK_OUT // 2  # matmul N-tiles per chunk (N=512 = 2 rows)
NTAP = KH * KW  # 9 kernel taps
PSUM_BUFS = 2

_cache = {}


def _build():
    import concourse.bacc as bacc
    import concourse.mybir as mybir
    from concourse.tile import TileContext

    BF16 = mybir.dt.bfloat16
    F32 = mybir.dt.float32

    nc = bacc.Bacc(name="condconv")
    x_d = nc.dram_tensor("xp", [SPC, CIN, PADROWS, RP], BF16, kind="ExternalInput")
    w_d = nc.dram_tensor("wt", [128, NTAP * 128], BF16, kind="ExternalInput")
    b_d = nc.dram_tensor("bias", [128, 1], F32, kind="ExternalInput")
    y_d = nc.dram_tensor("y", [SPC, COUT, HH, W], BF16, kind="ExternalOutput")

    with TileContext(nc) as tc:
        with (
            tc.tile_pool(name="strip", bufs=4) as strip_pool,
            tc.tile_pool(name="wtp", bufs=1) as wt_pool,
            tc.tile_pool(name="stage", bufs=6) as stage_pool,
            tc.tile_pool(name="psum", bufs=PSUM_BUFS, space="PSUM") as psum_pool,
        ):
            wt = wt_pool.tile([128, NTAP * 128], BF16)
            bias = wt_pool.tile([128, 1], F32)

            def load_strip(s):
                r0 = s * STRIP_OUT
                strip = strip_pool.tile([128, SFREE], BF16, name="strip", tag="strip")
                s3 = strip.rearrange("p (y u) -> p y u", y=STRIP_ROWS)
                # strip 0 gates kernel start: load it in row-pieces so the
                # first chunk's matmuls begin after ~10 rows, not all 34
                if s == 0:
                    # startup interleave on the serial DMA bus: piece 1 (rows
                    # 0-3, gates the first matmul) first, weight slices
                    # between the later pieces (tap p's matmuls only need
                    # wt[:, 512p:...], tracked at region granularity); bias
                    # (512B, needed by the first evac ~7us in) rides second
                    nc.sync.dma_start(out=s3[:, 0:4], in_=x_d[:, :, r0 : r0 + 4, :])
                    nc.sync.dma_start(out=bias, in_=b_d[:, :])
                    nc.sync.dma_start(out=wt[:, 0:512], in_=w_d[:, 0:512])
                    nc.sync.dma_start(
                        out=s3[:, 4:10], in_=x_d[:, :, r0 + 4 : r0 + 10, :]
                    )
                    nc.sync.dma_start(out=wt[:, 512:], in_=w_d[:, 512:])
                    nc.sync.dma_start(
                        out=s3[:, 10:18], in_=x_d[:, :, r0 + 10 : r0 + 18, :]
                    )
                    nc.sync.dma_start(
                        out=s3[:, 18:34], in_=x_d[:, :, r0 + 18 : r0 + 34, :]
                    )
                else:
                    # one fused DMA (partition dim = samples x cin): a single
                    # HWDGE generation instead of four serialized ones
                    nc.sync.dma_start(
                        out=s3[:, :],
                        in_=x_d[:, :, r0 : r0 + STRIP_ROWS, :],
                    )
                return s3

            def emit_chunk(
                s3, r0, t0, nrows, out_sp=False, evac_dve=False, stag="st"
            ):
                """One PSUM chunk: output rows t0..t0+nrows (strip-local)."""
                ntiles = (nrows + 1) // 2
                # uniform PSUM tile size so the 2-buf rotation fits 8 banks
                ps = psum_pool.tile([128, NT * 512], F32, name="ps", tag="ps")
                # tap-major: consecutive matmuls share each stationary
                for p in range(NTAP):
                    kh, kw = divmod(p, KW)
                    for nt in range(ntiles):
                        t = t0 + nt * 2
                        r = min(2, nrows - nt * 2)
                        nc.tensor.matmul(
                            ps[:, nt * 512 : nt * 512 + r * 256],
                            wt[:, p * 128 : (p + 1) * 128],
                            s3[:, t + kh : t + kh + r, kw : kw + W],
                            start=(p == 0),
                            stop=(p == NTAP - 1),
                        )
                fsz = nrows * 256
                stage = stage_pool.tile([128, fsz], BF16, name="st", tag="st")
                # PSUM->SBUF evac with fused per-partition bias add; the
                # drain path alternates ACT/DVE so consecutive evacs don't
                # serialize behind each other's output-DMA semaphores
                if evac_dve:
                    nc.vector.tensor_scalar_add(
                        out=stage, in0=ps[:, 0:fsz], scalar1=bias
                    )
                else:
                    nc.scalar.add(out=stage, in_=ps[:, 0:fsz], add=bias)
                lo = r0 + t0
                dma = {0: nc.gpsimd.dma_start, 1: nc.sync.dma_start,
                       2: nc.scalar.dma_start}[int(out_sp)]
                dma(
                    out=y_d[:, :, lo : lo + nrows, :],
                    in_=stage.rearrange("p (r w) -> p r w", r=nrows),
                )

            def emit_chunks(s, s3, first=False, last=False):
                r0 = s * STRIP_OUT
                for q in range(NCHUNK):
                    if first and q == 0:
                        # 2-row minis: each gated only on its own input rows,
                        # so matmuls start as soon as the 4-row piece lands
                        for t0 in range(0, CHUNK_OUT, 2):
                            emit_chunk(s3, r0, t0, 2)
                    elif last and q == NCHUNK - 1:
                        emit_chunk(s3, r0, q * CHUNK_OUT, 4)
                        emit_chunk(
                            s3, r0, q * CHUNK_OUT + 4, 4,
                            out_sp=True, evac_dve=True,
                        )
                    else:
                        emit_chunk(s3, r0, q * CHUNK_OUT, CHUNK_OUT)

            # PE warmup while strip 0 loads: throwaway matmuls on the weight
            # tile ramp the PE clock (0.65/1.2 GHz p-states -> 2.4 GHz after
            # ~3us of continuous busy) so real matmuls start at full speed
            # memset-sourced warmup: DVE fills a scratch tile in ~0.7us (no
            # DMA latency chain), so PE ramping starts at ~0.8us instead of
            # waiting ~3us for the first weight DMA to land
            warm = wt_pool.tile([128, 512], BF16)
            nc.vector.memset(warm, 0.0)
            wps = psum_pool.tile([128, NT * 512], F32, name="ps", tag="ps")
            NWARM = 5
            for i in range(NWARM):
                nc.tensor.matmul(
                    wps[:, 0:512],
                    warm[:, 0:128],
                    warm[:, :],
                    start=(i == 0),
                    stop=(i == NWARM - 1),
                )

            # software pipeline: strip s+1's loads land before strip s's
            # matmuls consume strip s
            pending = None
            for s in range(NSTRIPS):
                s3 = load_strip(s)
                if pending is not None:
                    emit_chunks(pending[0], pending[1], first=(pending[0] == 0))
                pending = (s, s3)
            emit_chunks(pending[0], pending[1], last=True)
    nc.compile()
    return nc


def kernel(x, routing_weights, expert_weight, expert_bias):
    import ml_dtypes
    from concourse import bass_utils

    bf16 = ml_dtypes.bfloat16
    x = np.ascontiguousarray(x, dtype=np.float32)
    routing_weights = np.asarray(routing_weights, dtype=np.float32)
    expert_weight = np.asarray(expert_weight, dtype=np.float32)
    expert_bias = np.asarray(expert_bias, dtype=np.float32)

    # Host: routing mix (trivial flops) + weight/input repacking.
    w_all = routing_weights @ expert_weight  # [B, COUT*CIN*KH*KW]
    bias = routing_weights @ expert_bias  # [B, COUT]
    w4 = w_all.reshape(B, COUT, CIN, KH, KW)

    # Per group of 4 samples: 9 block-diagonal stationaries, k-major layout
    # wtb[k=(s,ci), p=(kh,kw), m=(s,co)] = w4[sample s, co, ci, kh, kw]
    ngrp = B // SPC
    wtb = np.zeros((ngrp, 128, NTAP, 128), np.float32)
    # w4 grouped: [ngrp, SPC, COUT, CIN, KH, KW] -> [g, (s ci), (kh kw), co]
    w4g = w4.reshape(ngrp, SPC, COUT, CIN, KH, KW)
    for s in range(SPC):
        blk = w4g[:, s].transpose(0, 2, 3, 4, 1)  # [g, CIN, KH, KW, COUT]
        wtb[:, 32 * s : 32 * s + 32, :, 32 * s : 32 * s + 32] = blk.reshape(
            ngrp, CIN, NTAP, COUT
        )
    wtb = wtb.reshape(ngrp, 128, NTAP * 128).astype(bf16)
    biasb = bias.reshape(ngrp, SPC * COUT, 1).astype(np.float32)

    # Padded input: col c in 1..256 holds x col c-1; row r in 1..256 holds
    # x row r-1; everything else zero.
    xp = np.zeros((B, CIN, H + 2, RP), bf16)
    xp[:, :, 1 : 1 + H, 1 : 1 + W] = x

    if "nc" not in _cache:
        _cache["nc"] = _build()
    nc = _cache["nc"]

    # core c: sample group c//2, row half c%2 (row slab 128*h .. 128*h+129)
    in_maps = []
    for c in range(NCORES):
        g, h = divmod(c, 2)
        slab = xp[g * SPC : (g + 1) * SPC, :, h * HH : h * HH + PADROWS, :]
        in_maps.append(
            {
                "xp": np.ascontiguousarray(slab),
                "wt": wtb[g],
                "bias": biasb[g],
            }
        )
    import os

    trace = bool(int(os.environ.get("CONDCONV_TRACE", "0")))
    res = bass_utils.run_bass_kernel_spmd(
        nc, in_maps, core_ids=list(range(NCORES)), trace=trace
    )
    _cache["last_results"] = res
    y = np.empty((B, COUT, H, W), np.float32)
    for c in range(NCORES):
        g, h = divmod(c, 2)
        y[g * SPC : (g + 1) * SPC, :, h * HH : (h + 1) * HH, :] = res.results[c][
            "y"
        ].astype(np.float32)
    return y
